# revision 28
# baseline (speedup 1.0000x reference)
"""Trainium2 Bass kernel for the vq_codebook problem.

Computes, per batch b (B=32, d=512, n=4096, r=64, T=10, 3 steps):
    D = normalize(D_init, dim=d)
    repeat 3x: Dn = normalize(D); cos = Dn^T @ normalize(X, dim=d);
               C = softmax(cos / T, over r); D = X @ C^T   (normalize-invariant
               scale factors like the per-codeword count division cancel)
    Xbar = normalize(D) @ C of the last step.

Sharding: pure batch parallelism, 4 batches per NeuronCore across 8 cores.

Layout strategy per batch:
  - X loaded natural [d, n]; PE-transposed once to XT [n, d] (f32r) for the
    n-contraction (XCt); cast to bf16 for the d-contraction (cos).
  - All softmax work happens in the transposed [n-on-partitions, r-free]
    layout where the 1/||x_n|| logit scale and the softmax denominator are
    per-partition/free-dim ops.
  - Big matmuls use float32r (full PE rate at N>=256, ~1e-4 relative error).
"""

import numpy as np

import concourse.bacc as bacc
import concourse.bass as bass
import concourse.mybir as mybir
import concourse.tile as tile
from concourse.bass_utils import run_bass_kernel_spmd

F32 = mybir.dt.float32
F32R = mybir.dt.float32r
BF16 = mybir.dt.bfloat16
AF = mybir.ActivationFunctionType
OP = mybir.AluOpType

N_CORES = 8
B_FULL, D, N, R = 32, 512, 4096, 64
B_LOC = B_FULL // N_CORES          # 4 batches per core
KT = D // 128                      # 4 d-tiles
NC128 = N // 128                   # 32 n-chunks of 128
NB512 = N // 512                   # 8 n-blocks of 512
NG = NC128 // 8                    # 4 groups of 8 chunks (512 n each)
T_INV = 0.1                        # 1 / temperature
STEPS = 3
EPS2 = 1e-12                       # eps^2 for the norm clamp


def _bcast(ap_2d, free_rep):
    """View a [P, m] AP as [P, m, free_rep] with stride-0 inner dim."""
    return bass.AP(
        tensor=ap_2d.tensor,
        offset=ap_2d.offset,
        ap=[ap_2d.ap[0], list(ap_2d.ap[1]), [0, free_rep]],
    )


def _rsqrt_clamped(nc, pool, src_ap, p, name):
    """exp(-0.5 * ln(max(src, EPS2))) as an [p, m] tile; src_ap is [p, m]."""
    m = src_ap.shape[1]
    cl = pool.tile([p, m], F32, tag=f"{name}_cl")
    nc.vector.tensor_scalar_max(out=cl, in0=src_ap, scalar1=EPS2)
    ln = pool.tile([p, m], F32, tag=f"{name}_ln")
    nc.scalar.activation(out=ln, in_=cl, func=AF.Ln, scale=1.0, bias=0.0)
    rs = pool.tile([p, m], F32, tag=f"{name}_rs")
    nc.scalar.activation(out=rs, in_=ln, func=AF.Exp, scale=-0.5, bias=0.0)
    return rs


def _force_single_act_set():
    """All ACT functions we use (Exp, Ln, Square, Copy) live in the
    natural_log_exp_and_others set.  The table-load pass first-matches each
    function against the set list, which alternates loads between two sets
    (~1.3 us each).  Empty out every other set (ids keep their positions) so
    everything resolves to the one set and a single load suffices."""
    import concourse.hw_specs as hw_specs

    orig = hw_specs.get_activation_tables
    target = "natural_log_exp_and_others"

    def patched(arch):
        t = dict(orig(arch))
        need = {AF.Exp, AF.Ln, AF.Square, AF.Copy}
        if target in t and need <= set(t[target]):
            t = {k: (v if k == target else set()) for k, v in t.items()}
        return t

    bacc.get_activation_tables = patched


def build_program():
    _force_single_act_set()
    nc = bacc.Bacc()
    x_ext = nc.declare_dram_parameter("X", [B_LOC, D, N], F32, isOutput=False)
    d_ext = nc.declare_dram_parameter("Dinit", [B_LOC, D, R], F32, isOutput=False)
    id_ext = nc.declare_dram_parameter("ident", [128, 128], F32, isOutput=False)
    y_ext = nc.declare_dram_parameter("Y", [B_LOC, D, N], F32, isOutput=True)

    with tile.TileContext(nc) as tc:
        import contextlib

        with contextlib.ExitStack() as ctx:
            singles = ctx.enter_context(tc.tile_pool(name="singles", bufs=1))
            xpool = ctx.enter_context(tc.tile_pool(name="xpool", bufs=1))
            xnat = ctx.enter_context(tc.tile_pool(name="xnat", bufs=8))
            work = ctx.enter_context(tc.tile_pool(name="work", bufs=2))
            work3 = ctx.enter_context(tc.tile_pool(name="work3", bufs=3))
            dpool = ctx.enter_context(tc.tile_pool(name="dpool", bufs=2))
            ps_big = ctx.enter_context(tc.tile_pool(name="ps_big", bufs=2, space="PSUM"))
            ps_cos = ctx.enter_context(tc.tile_pool(name="ps_cos", bufs=2, space="PSUM"))
            ps_ct = ctx.enter_context(tc.tile_pool(name="ps_ct", bufs=2, space="PSUM"))
            ps_acc = ctx.enter_context(tc.tile_pool(name="ps_acc", bufs=1, space="PSUM"))

            # identities in the three matmul dtypes
            id_f = singles.tile([128, 128], F32)
            nc.sync.dma_start(out=id_f, in_=id_ext[:])
            id_b = singles.tile([128, 128], BF16)
            nc.vector.tensor_copy(out=id_b, in_=id_f)

            for b in range(B_LOC):
                # ---------------- setup: load X, transpose, cast, norms ------
                xbf = [xpool.tile([128, N], BF16, tag=f"xbf{k}", name=f"xbf{k}") for k in range(KT)]
                xt = [xpool.tile([128, D], BF16, tag=f"xt{c}", name=f"xt{c}", bufs=2) for c in range(NC128)]
                ssq = xpool.tile([128, NC128], F32, tag="ssq")

                for h in range(4):  # quarters of n
                    xn_h = []
                    for k in range(KT):
                        t = xnat.tile([128, N // 4], F32, tag="xnat")
                        nc.sync.dma_start(
                            out=t,
                            in_=x_ext[b, k * 128:(k + 1) * 128,
                                      h * (N // 4):(h + 1) * (N // 4)],
                        )
                        xn_h.append(t)
                        nc.vector.tensor_copy(
                            out=xbf[k][:, h * (N // 4):(h + 1) * (N // 4)], in_=t
                        )
                    for ci in range(NC128 // 4):
                        c = h * (NC128 // 4) + ci
                        pt = ps_big.tile([128, D], F32, tag="pbig")
                        for k in range(KT):
                            nc.tensor.transpose(
                                pt[:, k * 128:(k + 1) * 128],
                                xn_h[k][:, ci * 128:(ci + 1) * 128],
                                id_f,
                            )
                        nc.vector.tensor_copy(out=xt[c], in_=pt)
                        sq = ps_ct.tile([128, D], F32, tag="pct")
                        nc.scalar.activation(
                            out=sq, in_=pt, func=AF.Square, scale=1.0, bias=0.0,
                            accum_out=ssq[:, c:c + 1],
                        )
                # scl[p, c] = 1 / max(||x_n||, eps), n = c*128 + p
                scl = xpool.tile([128, NC128], F32, tag="scl")
                rs = _rsqrt_clamped(nc, work, ssq[:, :], 128, "sclw")
                nc.vector.tensor_copy(out=scl, in_=rs)

                # D_init^T: load natural, transpose to DT [64, 512]
                dt_cur = dpool.tile([64, D], F32, tag="dt")
                pdn = ps_cos.tile([64, 512], F32, tag="pcos")
                for k in range(KT):
                    dn_nat = work.tile([128, R], F32, tag="dload")
                    nc.sync.dma_start(
                        out=dn_nat, in_=d_ext[b, k * 128:(k + 1) * 128, :]
                    )
                    nc.tensor.transpose(
                        pdn[:, k * 128:(k + 1) * 128], dn_nat, id_f
                    )
                nc.scalar.copy(out=dt_cur, in_=pdn)

                # ---------------- 3 VQ steps --------------------------------
                for s in range(STEPS):
                    last = s == STEPS - 1
                    # normalize D columns (rows of DT) -> DnT, transpose -> Dn (bf16)
                    dscr = ps_cos.tile([64, D], F32, tag="pcos")
                    ssqd = work.tile([64, 1], F32, tag="ssqd")
                    nc.vector.scalar_tensor_tensor(
                        out=dscr, in0=dt_cur, scalar=1.0, in1=dt_cur,
                        op0=OP.mult, op1=OP.mult, accum_out=ssqd,
                    )
                    rnd = _rsqrt_clamped(nc, work, ssqd[:, :], 64, "rnd")
                    dnt = work.tile([64, D], F32, tag="dnt")
                    nc.vector.tensor_scalar_mul(out=dnt, in0=dt_cur, scalar1=rnd)
                    dn_bf = work.tile([128, KT, R], BF16, tag="dnbf")
                    pdn2 = ps_big.tile([128, KT * R], F32, tag="pbig")
                    for k in range(KT):
                        nc.tensor.transpose(
                            pdn2[:, k * R:(k + 1) * R],
                            dnt[:, k * 128:(k + 1) * 128], id_f[0:64, 0:64],
                        )
                    nc.scalar.copy(out=dn_bf, in_=pdn2.rearrange("p (k r) -> p k r", k=KT))

                    # cos blocks (col-tiled pairs), E-transpose (row-tiled
                    # pairs), softmax, CT.  Pair g covers n-blocks 2g (top
                    # half of the psum tile) and 2g+1 (bottom half).
                    ct_g = []
                    for g in range(NG):
                        pct = ps_ct.tile([128, 4, 128], F32, tag="pct")
                        pc2 = ps_cos.tile([128, 512], F32, tag="pcos")
                        j0, j1 = 2 * g, 2 * g + 1
                        for k in range(KT):
                            nc.tensor.matmul(
                                pc2[0:64, :], dn_bf[:, k, :],
                                xbf[k][:, j0 * 512:(j0 + 1) * 512],
                                start=(k == 0), stop=(k == KT - 1),
                                tile_position=(0, 0),
                            )
                            nc.tensor.matmul(
                                pc2[64:128, :], dn_bf[:, k, :],
                                xbf[k][:, j1 * 512:(j1 + 1) * 512],
                                start=(k == 0), stop=(k == KT - 1),
                                tile_position=(0, 64), skip_group_check=True,
                            )
                        cos_sb = work3.tile([128, 512], F32, tag="cossb")
                        nc.scalar.copy(out=cos_sb, in_=pc2)
                        # One full 128x128 transpose flips a [2*r, n128]
                        # block: out columns 0:64 = cosT of block j0,
                        # 64:128 = cosT of block j1 (both at this n-chunk).
                        for ci in range(4):
                            nc.tensor.transpose(
                                pct[:, ci, :],
                                cos_sb[:, ci * 128:(ci + 1) * 128],
                                id_f,
                            )
                        # scale order along the packed axis: (ci, half) ->
                        # chunk (2g+half)*4+ci = scl column 8g + 4*half + ci
                        scl_s = scl[:, 8 * g:8 * (g + 1)]
                        scl_v = bass.AP(
                            tensor=scl_s.tensor, offset=scl_s.offset,
                            ap=[list(scl_s.ap[0]), [1, 4], [4, 2], [0, R]],
                        )
                        pct_v = pct.rearrange("p c (h r) -> p c h r", h=2)
                        logits = work3.tile([128, 4, 2, R], F32, tag="logits")
                        nc.vector.tensor_tensor(
                            out=logits, in0=pct_v, in1=scl_v, op=OP.mult,
                        )
                        et = work3.tile([128, 4, 2, R], F32, tag="et")
                        nc.scalar.activation(
                            out=et, in_=logits, func=AF.Exp, scale=T_INV, bias=0.0
                        )
                        s_sum = work3.tile([128, 4, 2], F32, tag="ssum")
                        nc.vector.tensor_reduce(
                            out=s_sum, in_=et, axis=mybir.AxisListType.X, op=OP.add
                        )
                        rs_sum = work3.tile([128, 4, 2], F32, tag="rssum")
                        nc.vector.reciprocal(out=rs_sum, in_=s_sum)
                        rs_b = bass.AP(
                            tensor=rs_sum.tensor, offset=rs_sum.offset,
                            ap=[list(rs_sum.ap[0]), [2, 4], [1, 2], [0, R]],
                        )
                        ct = work.tile([128, 4, 2, R], BF16, tag="ct", bufs=4, name=f"ct{g}")
                        nc.vector.tensor_tensor(
                            out=ct, in0=et, in1=rs_b, op=OP.mult
                        )
                        ct_g.append(ct)

                    # XCt^T [r=64, d=512]: bf16 col-tiled pairs — even chunks
                    # accumulate into partitions 0-63, odd into 64-127,
                    # halves summed after.  ct chunk for global chunk c is
                    # ct_g[c//8][:, c%4, (c%8)//4, :].
                    def ct_chunk(c):
                        return ct_g[c // 8][:, c % 4, (c % 8) // 4, :]

                    pacc = ps_acc.tile([128, D], F32, tag="pacc")
                    for cp in range(NC128 // 2):
                        ca, cb = 2 * cp, 2 * cp + 1
                        nc.tensor.matmul(
                            pacc[0:64, :], ct_chunk(ca), xt[ca],
                            start=(cp == 0), stop=(cp == NC128 // 2 - 1),
                            tile_position=(0, 0),
                        )
                        nc.tensor.matmul(
                            pacc[64:128, :], ct_chunk(cb), xt[cb],
                            start=(cp == 0), stop=(cp == NC128 // 2 - 1),
                            tile_position=(0, 64), skip_group_check=True,
                        )
                    xct_half = work.tile([64, D], F32, tag="xcthalf")
                    nc.scalar.copy(out=xct_half, in_=pacc[0:64, :])

                    if not last:
                        dt_cur = dpool.tile([64, D], F32, tag="dt")
                        nc.vector.tensor_tensor(
                            out=dt_cur, in0=xct_half, in1=pacc[64:128, :],
                            op=OP.add,
                        )
                    else:
                        # Dnew^T normalized, in bf16 for the Xbar matmul
                        dnew_f = work.tile([64, D], F32, tag="dnewf")
                        nc.vector.tensor_tensor(
                            out=dnew_f, in0=xct_half, in1=pacc[64:128, :],
                            op=OP.add,
                        )
                        fscr = ps_cos.tile([64, D], F32, tag="pcos")
                        ssqf = work.tile([64, 1], F32, tag="ssqf")
                        nc.vector.scalar_tensor_tensor(
                            out=fscr, in0=dnew_f, scalar=1.0, in1=dnew_f,
                            op0=OP.mult, op1=OP.mult, accum_out=ssqf,
                        )
                        rnf = _rsqrt_clamped(nc, work, ssqf[:, :], 64, "rnf")
                        dnew_r = work.tile([64, D], BF16, tag="dnewr")
                        nc.vector.tensor_scalar_mul(
                            out=dnew_r, in0=dnew_f, scalar1=rnf
                        )
                        # C [r=64, n] in bf16 via transposing CT chunks
                        c_r = xpool.tile([64, N], BF16, tag="c_r")
                        for q in range(NB512):
                            pcq = ps_cos.tile([64, 512], BF16, tag="pcos")
                            for ci in range(4):
                                c = q * 4 + ci
                                nc.tensor.transpose(
                                    pcq[:, ci * 128:(ci + 1) * 128],
                                    ct_chunk(c), id_b,
                                )
                            nc.vector.tensor_copy(
                                out=c_r[:, q * 512:(q + 1) * 512], in_=pcq
                            )
                        # Xbar = Dnew @ C
                        for k in range(KT):
                            for j in range(NB512):
                                pxb = ps_big.tile([128, 512], F32, tag="pbig")
                                nc.tensor.matmul(
                                    pxb, dnew_r[:, k * 128:(k + 1) * 128],
                                    c_r[:, j * 512:(j + 1) * 512],
                                    start=True, stop=True,
                                )
                                ot = work3.tile([128, 512], F32, tag="osb")
                                nc.scalar.copy(out=ot, in_=pxb)
                                nc.sync.dma_start(
                                    out=y_ext[b, k * 128:(k + 1) * 128,
                                              j * 512:(j + 1) * 512],
                                    in_=ot,
                                )
    nc.finalize()
    return nc


_NC_CACHE = None
_last_in_maps = None


def kernel(X: np.ndarray, D_init: np.ndarray) -> np.ndarray:
    global _NC_CACHE, _last_in_maps
    if _NC_CACHE is None:
        _NC_CACHE = build_program()
    nc = _NC_CACHE
    ident = np.eye(128, dtype=np.float32)
    in_maps = [
        {
            "X": np.ascontiguousarray(X[i * B_LOC:(i + 1) * B_LOC]),
            "Dinit": np.ascontiguousarray(D_init[i * B_LOC:(i + 1) * B_LOC]),
            "ident": ident,
        }
        for i in range(N_CORES)
    ]
    _last_in_maps = in_maps
    res = run_bass_kernel_spmd(nc, in_maps, list(range(N_CORES)))
    return np.concatenate([res.results[i]["Y"] for i in range(N_CORES)], axis=0)
